# revision 1
# baseline (speedup 1.0000x reference)
# Per-sample channel affine (color calibration): out = w[b,c] * image[b,c,h,w] + b[b,c]
# where w/b come from gathering tiny per-camera / per-identity tables.
#
# Strategy: pure data-parallel over the batch dim across 8 NeuronCores
# (4 samples = 12 image planes of 4 MiB per core). The table gather is a
# [32,3] host-side numpy op; the device kernel streams the 402 MB image
# through SBUF with a fused scale+bias (DVE tensor_scalar) per plane.
# Loads issue on the SP HWDGE ring, stores on the ACT ring, so store
# semaphore waits never stall load prefetch.
import numpy as np

import concourse.bacc as bacc
import concourse.bass as bass
import concourse.mybir as mybir
import concourse.tile as tile
from concourse.bass_utils import run_bass_kernel_spmd

N_CORES = 8
B, C, H, W = 32, 3, 1024, 1024
BPC = B // N_CORES          # samples per core
PLANES = BPC * C            # image planes per core
P = 128                     # SBUF partitions
COLS = H * W // P           # free-dim elements per plane tile
BUFS = 5

TRACE = False               # test.py flips this to collect NTFF exec time
LAST_RESULTS = None

_NC = None


def _build():
    nc = bacc.Bacc(
        "TRN2",
        target_bir_lowering=False,
        debug=False,
        enable_asserts=True,
        num_devices=1,
    )
    x = nc.dram_tensor("x", [PLANES, P, COLS], mybir.dt.float32, kind="ExternalInput").ap()
    wb = nc.dram_tensor("wb", [P, 2 * PLANES], mybir.dt.float32, kind="ExternalInput").ap()
    y = nc.dram_tensor("y", [PLANES, P, COLS], mybir.dt.float32, kind="ExternalOutput").ap()

    with tile.TileContext(nc) as tc:
        with (
            tc.tile_pool(name="const", bufs=1) as cpool,
            tc.tile_pool(name="data", bufs=BUFS) as pool,
        ):
            wb_sb = cpool.tile([P, 2 * PLANES], mybir.dt.float32)
            nc.sync.dma_start(wb_sb[:], wb[:])
            for j in range(PLANES):
                t = pool.tile([P, COLS], mybir.dt.float32, tag="plane")
                nc.sync.dma_start(t[:], x[j, :, :])
                nc.vector.tensor_scalar(
                    t[:],
                    t[:],
                    wb_sb[:, j : j + 1],
                    wb_sb[:, PLANES + j : PLANES + j + 1],
                    mybir.AluOpType.mult,
                    mybir.AluOpType.add,
                )
                nc.scalar.dma_start(y[j, :, :], t[:])
    nc.compile()
    return nc


def kernel(image, camindex, idindex, wcam, bcam, wident, bident):
    global _NC, LAST_RESULTS
    image = np.ascontiguousarray(np.asarray(image), dtype=np.float32)
    camindex = np.asarray(camindex).astype(np.int64)
    idindex = np.asarray(idindex).astype(np.int64)
    wcam = np.asarray(wcam, dtype=np.float32)
    bcam = np.asarray(bcam, dtype=np.float32)
    wident = np.asarray(wident, dtype=np.float32)
    bident = np.asarray(bident, dtype=np.float32)

    w = wcam[camindex] + wident[idindex]    # [B, 3] fp32
    b = bcam[camindex] + bident[idindex]    # [B, 3] fp32

    if _NC is None:
        _NC = _build()

    in_maps = []
    for c in range(N_CORES):
        sl = slice(c * BPC, (c + 1) * BPC)
        x = image[sl].reshape(PLANES, P, COLS)
        wb = np.empty((P, 2 * PLANES), np.float32)
        wb[:, :PLANES] = w[sl].reshape(PLANES)[None, :]
        wb[:, PLANES:] = b[sl].reshape(PLANES)[None, :]
        in_maps.append({"x": x, "wb": wb})

    res = run_bass_kernel_spmd(
        _NC, in_maps, core_ids=list(range(N_CORES)), trace=TRACE
    )
    LAST_RESULTS = res
    return np.concatenate(
        [r["y"].reshape(BPC, C, H, W) for r in res.results], axis=0
    )
